# revision 11
# baseline (speedup 1.0000x reference)
"""GATv2 (DeepGraphConvLayer) Trainium2 kernel, 8 NeuronCores.

Strategy (node-major, per-edge matmul):
  Host: sort nodes by in-degree, pack into 128-node tiles (degree-sorted so
  per-tile padding to the max degree is small), deal tiles round-robin to
  8 cores (identical per-tile slot schedule on every core - the SPMD program
  is shared).  For every (tile, slot) the host materializes the transposed
  feature columns of the slot's source nodes, so the device computes
  x = W_src^T phi_src + W_dst^T phi_dst + b  directly with matmuls - no
  device gather is needed at all.
  Device (per tile): 3 matmuls per slot accumulate x into PSUM ([128, 260],
  4 extra columns carry the linear term 0.2*sum_d attn*x of the leaky-relu
  decomposition attn.LR(x) = 0.2*attn.x + 0.8*attn.relu(x)).  ACT computes
  relu and a bf16 copy of x, DVE computes the attention logits, exp runs on
  ACT, and the weighted message sum V = sum_j ex_j * x_j accumulates with
  scalar_tensor_tensor (2 heads on DVE, 2 on GPSIMD).  The message term is
  recovered as U = V - s * fd (linearity), rst = U/s + residual.  BatchNorm
  statistics accumulate on PE into PSUM, are all-reduced across the 8 cores
  with a collective, and a final normalize+relu pass writes the output.
"""

import sys

for _p in ("/opt/trn_rl_repo", "/root/.axon_site/_ro/trn_rl_repo"):
    if _p not in sys.path:
        sys.path.append(_p)

import numpy as np
import ml_dtypes

import concourse.bacc as bacc
import concourse.bass as bass
import concourse.tile as tile
from concourse import mybir
from concourse.bass_utils import run_bass_kernel_spmd

BF16 = ml_dtypes.bfloat16
F32 = mybir.dt.float32
BF = mybir.dt.bfloat16
AF = mybir.ActivationFunctionType
OP = mybir.AluOpType

P = 128
NCORES = 8
HEADS = 4
HEAD_DIM = 64
D_IN = HEADS * HEAD_DIM          # 256
D_AUG = D_IN + HEADS             # 260
NEG_SLOPE = 0.2
BN_EPS = 1e-5

_COMPILE_CACHE = {}


# --------------------------------------------------------------------------
# device program
# --------------------------------------------------------------------------

def build_program(T, d_sched, S):
    """T tiles per core, d_sched[t] slots in tile t, S = sum(d_sched)."""
    nc = bacc.Bacc("TRN2", target_bir_lowering=False, debug=False,
                   num_devices=NCORES)

    fpe = nc.dram_tensor("fpe", [P, S * 256], mybir.dt.bfloat16,
                         kind="ExternalInput")
    fown = nc.dram_tensor("fown", [P, T * 256], mybir.dt.bfloat16,
                          kind="ExternalInput")
    resid = nc.dram_tensor("resid", [P, T * 256], F32, kind="ExternalInput")
    maskt = nc.dram_tensor("maskt", [P, max(S, 1)], F32, kind="ExternalInput")
    wsrc = nc.dram_tensor("wsrc", [P, 2 * D_AUG], mybir.dt.bfloat16,
                          kind="ExternalInput")
    wdst = nc.dram_tensor("wdst", [P, 2 * D_AUG], mybir.dt.bfloat16,
                          kind="ExternalInput")
    bvec = nc.dram_tensor("bvec", [1, D_AUG], mybir.dt.bfloat16,
                          kind="ExternalInput")
    ones1 = nc.dram_tensor("ones1", [1, P], mybir.dt.bfloat16,
                           kind="ExternalInput")
    ones1f = nc.dram_tensor("ones1f", [1, P], F32, kind="ExternalInput")
    onesc = nc.dram_tensor("onesc", [P, 1], F32, kind="ExternalInput")
    attn8 = nc.dram_tensor("attn8", [P, 256], mybir.dt.bfloat16,
                           kind="ExternalInput")
    ident = nc.dram_tensor("ident", [P, P], mybir.dt.bfloat16,
                           kind="ExternalInput")
    gb = nc.dram_tensor("gb", [1, 512], F32, kind="ExternalInput")
    nweight = nc.dram_tensor("nweight", [1, 1], F32, kind="ExternalInput")

    aout = nc.dram_tensor("aout", [P, max(S, 1) * HEADS], F32,
                          kind="ExternalOutput")
    outp = nc.dram_tensor("outp", [P, T * 256], F32, kind="ExternalOutput")

    with tile.TileContext(nc) as tc:
        with tc.tile_pool(name="const", bufs=1) as cpool, \
             tc.tile_pool(name="big", bufs=1) as bigpool, \
             tc.tile_pool(name="io", bufs=2) as iopool, \
             tc.tile_pool(name="work", bufs=3) as wpool, \
             tc.tile_pool(name="psx", bufs=3, space="PSUM") as psx, \
             tc.tile_pool(name="psfd", bufs=2, space="PSUM") as psfd, \
             tc.tile_pool(name="psbn", bufs=1, space="PSUM") as psbn, \
             tc.tile_pool(name="dram", bufs=1, space="DRAM") as dpool:

            # ---- constants ----
            wsrc_sb = cpool.tile([P, 2 * D_AUG], BF)
            nc.sync.dma_start(out=wsrc_sb[:], in_=wsrc[:, :])
            wdst_sb = cpool.tile([P, 2 * D_AUG], BF)
            nc.sync.dma_start(out=wdst_sb[:], in_=wdst[:, :])
            bvec_sb = cpool.tile([1, D_AUG], BF)
            nc.sync.dma_start(out=bvec_sb[:], in_=bvec[:, :])
            ones1_sb = cpool.tile([1, P], BF)
            nc.sync.dma_start(out=ones1_sb[:], in_=ones1[:, :])
            ones1f_sb = cpool.tile([1, P], F32)
            nc.sync.dma_start(out=ones1f_sb[:], in_=ones1f[:, :])
            onesc_sb = cpool.tile([P, 1], F32)
            nc.sync.dma_start(out=onesc_sb[:], in_=onesc[:, :])
            attn8_sb = cpool.tile([P, 256], BF)
            nc.sync.dma_start(out=attn8_sb[:], in_=attn8[:, :])
            ident_sb = cpool.tile([P, P], BF)
            nc.sync.dma_start(out=ident_sb[:], in_=ident[:, :])
            gb_sb = cpool.tile([1, 512], F32)
            nc.sync.dma_start(out=gb_sb[:], in_=gb[:, :])
            nw_sb = cpool.tile([1, 1], F32)
            nc.sync.dma_start(out=nw_sb[:], in_=nweight[:, :])

            # persistent rst storage (f32, one 256-block per tile)
            rst_sb = bigpool.tile([P, T * 256], F32)

            # separate PSUM tiles: a matmul accumulation group's start=True
            # clears the whole bank, so sum and sumsq must not share one
            bn_sum = psbn.tile([1, 256], F32, tag="bnsum")
            bn_sq = psbn.tile([1, 256], F32, tag="bnsq")

            off = 0  # slot offset
            for t in range(T):
                D = d_sched[t]

                # ---- tile loads ----
                fown_sb = iopool.tile([P, 256], BF, tag="fown")
                nc.sync.dma_start(out=fown_sb[:],
                                  in_=fown[:, t * 256:(t + 1) * 256])
                resid_sb = iopool.tile([P, 256], F32, tag="resid")
                nc.sync.dma_start(out=resid_sb[:],
                                  in_=resid[:, t * 256:(t + 1) * 256])
                if D > 0:
                    fpe_sb = iopool.tile([P, D * 256], BF, tag="fpe")
                    nc.sync.dma_start(out=fpe_sb[:],
                                      in_=fpe[:, off * 256:(off + D) * 256])
                    mask_sb = iopool.tile([P, D], F32, tag="mask")
                    nc.sync.dma_start(out=mask_sb[:],
                                      in_=maskt[:, off:off + D])

                # ---- fd' = W_dst^T phi_own + bias (+ A-columns) ----
                fd_ps = psfd.tile([P, D_AUG], F32)
                nc.tensor.matmul(out=fd_ps[:], lhsT=fown_sb[:, 0:128],
                                 rhs=wdst_sb[:, 0:D_AUG],
                                 start=True, stop=False)
                nc.tensor.matmul(out=fd_ps[:], lhsT=fown_sb[:, 128:256],
                                 rhs=wdst_sb[:, D_AUG:2 * D_AUG],
                                 start=False, stop=False)
                nc.tensor.matmul(out=fd_ps[:], lhsT=ones1_sb[:],
                                 rhs=bvec_sb[:], start=False, stop=True)
                fd_sb = wpool.tile([P, D_AUG], BF, tag="fd")
                nc.vector.tensor_copy(out=fd_sb[:], in_=fd_ps[:])

                if D > 0:
                    ex_buf = wpool.tile([P, HEADS * D], F32, tag="exbuf")
                    v01 = wpool.tile([P, 128], F32, tag="v01")
                    v23 = wpool.tile([P, 128], F32, tag="v23")
                    nc.gpsimd.memset(v01[:], 0.0)
                    nc.gpsimd.memset(v23[:], 0.0)

                for j in range(D):
                    # x = fs_j + fd' into PSUM [128, 260]
                    x_ps = psx.tile([P, D_AUG], F32, tag="x")
                    nc.tensor.matmul(out=x_ps[:], lhsT=ident_sb[:],
                                     rhs=fd_sb[:], start=True, stop=False)
                    base = (off + j) * 256
                    nc.tensor.matmul(out=x_ps[:],
                                     lhsT=fpe_sb[:, j * 256:j * 256 + 128],
                                     rhs=wsrc_sb[:, 0:D_AUG],
                                     start=False, stop=False)
                    nc.tensor.matmul(out=x_ps[:],
                                     lhsT=fpe_sb[:, j * 256 + 128:(j + 1) * 256],
                                     rhs=wsrc_sb[:, D_AUG:2 * D_AUG],
                                     start=False, stop=True)

                    # relu(x) and bf16 copy of x (ACT)
                    r_sb = wpool.tile([P, 256], BF, tag="r")
                    nc.scalar.activation(out=r_sb[:], in_=x_ps[:, 0:256],
                                         func=AF.Relu)
                    x_sb = wpool.tile([P, 256], BF, tag="xs")
                    nc.scalar.activation(out=x_sb[:], in_=x_ps[:, 0:256],
                                         func=AF.Copy)

                    # logits = 0.8 * sum_d attn*relu + (0.2*A from psum cols)
                    w_sb = wpool.tile([P, 256], BF, tag="w")
                    nc.gpsimd.tensor_tensor(out=w_sb[:], in0=r_sb[:],
                                            in1=attn8_sb[:], op=OP.mult)
                    red_sb = wpool.tile([P, HEADS], F32, tag="red")
                    nc.vector.tensor_reduce(
                        out=red_sb[:],
                        in_=w_sb[:].rearrange("p (g s) -> p g s", s=HEAD_DIM),
                        axis=mybir.AxisListType.X, op=OP.add)
                    logit_sb = wpool.tile([P, HEADS], F32, tag="logit")
                    nc.vector.tensor_tensor(out=logit_sb[:], in0=red_sb[:],
                                            in1=x_ps[:, 256:D_AUG],
                                            op=OP.add)

                    # ex = exp(logit) * mask  -> ex_buf (head-major)
                    extmp = wpool.tile([P, HEADS], F32, tag="extmp")
                    nc.scalar.activation(out=extmp[:], in_=logit_sb[:],
                                         func=AF.Exp)
                    exview = ex_buf[:].rearrange("p (h j) -> p h j",
                                                 h=HEADS)[:, :, j:j + 1]
                    nc.vector.tensor_scalar(
                        out=exview,
                        in0=extmp[:].rearrange("p (h o) -> p h o", o=1),
                        scalar1=mask_sb[:, j:j + 1], scalar2=None,
                        op0=OP.mult)

                    # V += ex * x  (per head, DVE; GPSIMD lacks the op)
                    for h in range(HEADS):
                        vdst = v01 if h < 2 else v23
                        vs = (h % 2) * 64
                        eng = nc.vector
                        eng.scalar_tensor_tensor(
                            out=vdst[:, vs:vs + 64],
                            in0=x_sb[:, h * 64:(h + 1) * 64],
                            scalar=ex_buf[:, h * D + j:h * D + j + 1],
                            in1=vdst[:, vs:vs + 64],
                            op0=OP.mult, op1=OP.add)

                # ---- tile epilogue: softmax denom, a, rst ----
                rst_t = rst_sb[:, t * 256:(t + 1) * 256]
                if D > 0:
                    s_sb = wpool.tile([P, HEADS], F32, tag="s")
                    nc.vector.tensor_reduce(
                        out=s_sb[:],
                        in_=ex_buf[:].rearrange("p (h j) -> p h j", h=HEADS),
                        axis=mybir.AxisListType.X, op=OP.add)
                    # negs from the raw s (so empty rows give U = V = 0),
                    # epsilon only in the reciprocal
                    negs = wpool.tile([P, HEADS], F32, tag="negs")
                    nc.vector.tensor_scalar(out=negs[:], in0=s_sb[:],
                                            scalar1=-1.0, scalar2=None,
                                            op0=OP.mult)
                    nc.vector.tensor_scalar(out=s_sb[:], in0=s_sb[:],
                                            scalar1=1e-30, scalar2=None,
                                            op0=OP.add)
                    rre = wpool.tile([P, HEADS], F32, tag="rre")
                    nc.vector.reciprocal(out=rre[:], in_=s_sb[:])

                    # attention output a = ex / s
                    a_sb = wpool.tile([P, HEADS * D], F32, tag="a")
                    nc.vector.tensor_tensor(
                        out=a_sb[:].rearrange("p (h j) -> p h j", h=HEADS),
                        in0=ex_buf[:].rearrange("p (h j) -> p h j", h=HEADS),
                        in1=rre[:].rearrange("p (h o) -> p h o", o=1)
                            .to_broadcast([P, HEADS, D]),
                        op=OP.mult)
                    nc.sync.dma_start(
                        out=aout[:, off * HEADS:(off + D) * HEADS],
                        in_=a_sb[:])

                    # rst = (V - s*fd') / s + resid
                    for h in range(HEADS):
                        vsrc = v01 if h < 2 else v23
                        vs = (h % 2) * 64
                        u_sb = wpool.tile([P, 64], F32, tag="u")
                        nc.vector.scalar_tensor_tensor(
                            out=u_sb[:],
                            in0=fd_sb[:, h * 64:(h + 1) * 64],
                            scalar=negs[:, h:h + 1],
                            in1=vsrc[:, vs:vs + 64],
                            op0=OP.mult, op1=OP.add)
                        nc.vector.scalar_tensor_tensor(
                            out=rst_t[:, h * 64:(h + 1) * 64],
                            in0=u_sb[:],
                            scalar=rre[:, h:h + 1],
                            in1=resid_sb[:, h * 64:(h + 1) * 64],
                            op0=OP.mult, op1=OP.add)
                else:
                    nc.vector.tensor_copy(out=rst_t, in_=resid_sb[:])

                # ---- BN partial sums on PE ----
                sq_sb = wpool.tile([P, 256], F32, tag="sq")
                nc.vector.tensor_tensor(out=sq_sb[:], in0=rst_t, in1=rst_t,
                                        op=OP.mult)
                nc.tensor.matmul(out=bn_sum[:], lhsT=onesc_sb[:],
                                 rhs=rst_t, start=(t == 0), stop=(t == T - 1))
                nc.tensor.matmul(out=bn_sq[:], lhsT=onesc_sb[:],
                                 rhs=sq_sb[:], start=(t == 0),
                                 stop=(t == T - 1))
                off += D

            # ---- BN all-reduce + stats ----
            bn_row = wpool.tile([1, 512], F32, tag="bnrow")
            nc.vector.tensor_copy(out=bn_row[:, 0:256], in_=bn_sum[:])
            nc.vector.tensor_copy(out=bn_row[:, 256:512], in_=bn_sq[:])
            cin = dpool.tile([1, 512], F32)
            cout = dpool.tile([1, 512], F32)
            nc.gpsimd.dma_start(out=cin[:], in_=bn_row[:])
            nc.gpsimd.collective_compute(
                "AllReduce", OP.add,
                replica_groups=[list(range(NCORES))],
                ins=[cin[:].opt()], outs=[cout[:].opt()])
            red_row = wpool.tile([1, 512], F32, tag="redrow")
            nc.gpsimd.dma_start(out=red_row[:], in_=cout[:])

            stats = wpool.tile([1, 512], F32, tag="stats")
            # stats[0:256] = scale, stats[256:512] = shift
            mean = wpool.tile([1, 256], F32, tag="mean")
            nc.vector.tensor_scalar(out=mean[:], in0=red_row[:, 0:256],
                                    scalar1=nw_sb[:1, :1], scalar2=None,
                                    op0=OP.mult)
            ex2 = wpool.tile([1, 256], F32, tag="ex2")
            nc.vector.tensor_scalar(out=ex2[:], in0=red_row[:, 256:512],
                                    scalar1=nw_sb[:1, :1], scalar2=None,
                                    op0=OP.mult)
            var = wpool.tile([1, 256], F32, tag="var")
            nc.vector.tensor_tensor(out=var[:], in0=mean[:], in1=mean[:],
                                    op=OP.mult)
            nc.vector.tensor_tensor(out=var[:], in0=ex2[:], in1=var[:],
                                    op=OP.subtract)
            nc.vector.tensor_scalar(out=var[:], in0=var[:], scalar1=BN_EPS,
                                    scalar2=None, op0=OP.add)
            std = wpool.tile([1, 256], F32, tag="std")
            nc.scalar.activation(out=std[:], in_=var[:], func=AF.Sqrt)
            rstd = wpool.tile([1, 256], F32, tag="rstd")
            nc.vector.reciprocal(out=rstd[:], in_=std[:])
            # one Newton step for 1/sqrt: y <- y*(1.5 - 0.5*v*y^2)
            y2 = wpool.tile([1, 256], F32, tag="y2")
            nc.vector.tensor_tensor(out=y2[:], in0=rstd[:], in1=rstd[:],
                                    op=OP.mult)
            nc.vector.tensor_tensor(out=y2[:], in0=var[:], in1=y2[:],
                                    op=OP.mult)
            nc.vector.tensor_scalar(out=y2[:], in0=y2[:], scalar1=-0.5,
                                    scalar2=1.5, op0=OP.mult, op1=OP.add)
            nc.vector.tensor_tensor(out=rstd[:], in0=rstd[:], in1=y2[:],
                                    op=OP.mult)
            nc.vector.tensor_tensor(out=stats[:, 0:256], in0=rstd[:],
                                    in1=gb_sb[:, 0:256], op=OP.mult)
            ms = wpool.tile([1, 256], F32, tag="ms")
            nc.vector.tensor_tensor(out=ms[:], in0=mean[:],
                                    in1=stats[:, 0:256], op=OP.mult)
            nc.vector.tensor_tensor(out=stats[:, 256:512],
                                    in0=gb_sb[:, 256:512], in1=ms[:],
                                    op=OP.subtract)

            # broadcast scale/shift to 128 partitions via PE outer product
            bc_ps = psbn.tile([P, 512], F32, tag="bc")
            nc.tensor.matmul(out=bc_ps[:], lhsT=ones1f_sb[:], rhs=stats[:],
                             start=True, stop=True)
            bc_sb = bigpool.tile([P, 512], F32)
            nc.vector.tensor_copy(out=bc_sb[:], in_=bc_ps[:])

            # ---- phase 3: normalize + relu + store ----
            import os as _os
            dbg_rst = bool(_os.environ.get("BASS_GAT_RST"))
            for t in range(T):
                rst_t = rst_sb[:, t * 256:(t + 1) * 256]
                if dbg_rst:
                    nc.sync.dma_start(out=outp[:, t * 256:(t + 1) * 256],
                                      in_=rst_t)
                    continue
                tmp = wpool.tile([P, 256], F32, tag="ph3")
                nc.vector.tensor_tensor(out=tmp[:], in0=rst_t,
                                        in1=bc_sb[:, 0:256], op=OP.mult)
                nc.vector.tensor_tensor(out=tmp[:], in0=tmp[:],
                                        in1=bc_sb[:, 256:512], op=OP.add)
                o_sb = wpool.tile([P, 256], F32, tag="osb")
                nc.scalar.activation(out=o_sb[:], in_=tmp[:], func=AF.Relu)
                nc.sync.dma_start(out=outp[:, t * 256:(t + 1) * 256],
                                  in_=o_sb[:])

    nc.compile()
    return nc


# --------------------------------------------------------------------------
# host side
# --------------------------------------------------------------------------

def _prep(features, src, dst, W_src, b_src, W_dst, b_dst, attn_w, bias,
          gamma, beta):
    N, F = features.shape
    E = src.shape[0]
    src = np.asarray(src).astype(np.int64)
    dst = np.asarray(dst).astype(np.int64)

    deg = np.bincount(dst, minlength=N).astype(np.int64)
    order = np.argsort(-deg, kind="stable")  # nodes by degree desc

    T_ALL = -(-N // P)
    T_ALL = -(-T_ALL // NCORES) * NCORES      # multiple of 8
    n_pad = T_ALL * P - N
    nodes_padded = np.concatenate([order, np.zeros(n_pad, np.int64)])
    node_tiles = nodes_padded.reshape(T_ALL, P)      # [T_ALL, 128]
    is_pad = np.zeros(T_ALL * P, bool)
    is_pad[N:] = True
    pad_tiles = is_pad.reshape(T_ALL, P)

    # edge id matrix per node: eid_mat[n, r] = id of the r-th edge with dst=n
    maxd = int(deg.max()) if E else 1
    e_order = np.argsort(dst, kind="stable")
    dst_sorted = dst[e_order]
    starts = np.searchsorted(dst_sorted, np.arange(N))
    rank = np.arange(E, dtype=np.int64) - starts[dst_sorted]
    eid_mat = np.full((N, maxd), -1, np.int64)
    eid_mat[dst_sorted, rank] = e_order

    # per-tile degree = degree of first node (deg-sorted => max in tile)
    tile_deg = deg[node_tiles[:, 0]].copy()
    tile_deg[pad_tiles[:, 0]] = 0
    # stripe k = tiles [k*8, k*8+8); all cores share D_k = stripe max
    T = T_ALL // NCORES
    d_sched = [int(tile_deg[k * NCORES]) for k in range(T)]
    S = int(sum(d_sched))

    FT = np.ascontiguousarray(features.T)            # [256, N] f32
    FT_bf = FT.astype(BF16)

    battn = np.asarray(attn_w, np.float64)           # [4, 64]
    bsum = (np.asarray(b_src, np.float64) + np.asarray(b_dst, np.float64))

    def make_waug(W):
        W = np.asarray(W, np.float64)                # [256, 260]
        acols = 0.2 * np.stack(
            [W[:, h * 64:(h + 1) * 64] @ battn[h] for h in range(HEADS)],
            axis=1)
        return np.concatenate([W, acols], axis=1)

    waug_s = make_waug(W_src)
    waug_d = make_waug(W_dst)
    bcols = 0.2 * np.array(
        [battn[h] @ bsum[h * 64:(h + 1) * 64] for h in range(HEADS)])
    bv = np.concatenate([bsum, bcols])[None, :].astype(BF16)    # [1, 260]

    def chunks(W):        # [256, 260] -> [128, 2*260]
        return np.concatenate([W[0:128], W[128:256]], axis=1).astype(BF16)

    core_inputs = []
    core_meta = []
    for c in range(NCORES):
        tsel = np.arange(T) * NCORES + c
        nodes_c = node_tiles[tsel]                    # [T, 128]
        pad_c = pad_tiles[tsel]

        # per-tile slot data
        src_cols = np.zeros((T, P, max(S, 1)), np.int64)  # ragged via offs
        fpe_np = np.zeros((P, max(S, 1) * 256), BF16)
        mask_np = np.zeros((P, max(S, 1)), np.float32)
        eid_list = []
        offs = np.concatenate([[0], np.cumsum(d_sched)]).astype(np.int64)
        for t in range(T):
            D = d_sched[t]
            eid_list.append(None)
            if D == 0:
                continue
            nodes = nodes_c[t]
            eids = eid_mat[nodes, :D]                       # [128, D]
            valid = (eids >= 0) & (~pad_c[t][:, None])
            eids_v = np.where(valid, eids, 0)
            srcs = np.where(valid, src[eids_v], 0)          # [128, D]
            eid_list[t] = np.where(valid, eids_v, -1)
            mask_np[:, offs[t]:offs[t] + D] = valid.astype(np.float32)
            # G: [256, D*128] -> [128, D*256]
            g = FT_bf[:, srcs.T.reshape(-1)]                # [256, D*128]
            a4 = g.reshape(2, 128, D, 128).transpose(1, 2, 0, 3)
            fpe_np[:, offs[t] * 256:(offs[t] + D) * 256] = \
                a4.reshape(128, D * 256)

        nodes_flat = nodes_c.reshape(-1)
        g = FT_bf[:, nodes_flat]                            # [256, T*128]
        fown_np = np.ascontiguousarray(
            g.reshape(2, 128, T, 128).transpose(1, 2, 0, 3)
            .reshape(128, T * 256))

        resid_rows = features[nodes_flat] + np.asarray(bias, np.float32)
        resid_rows[pad_c.reshape(-1)] = 0.0
        resid_np = np.ascontiguousarray(
            resid_rows.reshape(T, P, 256).transpose(1, 0, 2)
            .reshape(P, T * 256).astype(np.float32))

        inp = {
            "fpe": fpe_np,
            "fown": fown_np,
            "resid": resid_np,
            "maskt": mask_np,
            "wsrc": chunks(waug_s),
            "wdst": chunks(waug_d),
            "bvec": bv,
            "ones1": np.ones((1, P), BF16),
            "ones1f": np.ones((1, P), np.float32),
            "onesc": np.ones((P, 1), np.float32),
            "attn8": np.tile((0.8 * battn.reshape(1, 256)).astype(BF16),
                             (P, 1)),
            "ident": np.eye(P, dtype=BF16),
            "gb": np.concatenate([np.asarray(gamma, np.float32),
                                  np.asarray(beta, np.float32)])[None, :],
            "nweight": np.full((1, 1), 1.0 / N, np.float32),
        }
        core_inputs.append(inp)
        core_meta.append({"nodes": nodes_c, "pad": pad_c, "eids": eid_list,
                          "offs": offs})

    return dict(T=T, d_sched=tuple(d_sched), S=S, core_inputs=core_inputs,
                core_meta=core_meta, N=N, E=E)


def _unshard(prep, results):
    N, E, T = prep["N"], prep["E"], prep["T"]
    d_sched = prep["d_sched"]
    out_full = np.zeros((N, 256), np.float32)
    a_full = np.zeros((E, HEADS), np.float32)
    for c in range(NCORES):
        meta = prep["core_meta"][c]
        outp = results[c]["outp"]          # [128, T*256]
        aout = results[c]["aout"]          # [128, S*4]
        offs = meta["offs"]
        for t in range(T):
            nodes = meta["nodes"][t]
            keep = ~meta["pad"][t]
            out_full[nodes[keep]] = outp[keep, t * 256:(t + 1) * 256]
            D = d_sched[t]
            if D == 0:
                continue
            eids = meta["eids"][t]                    # [128, D], -1 pad
            blk = aout[:, offs[t] * HEADS:(offs[t] + D) * HEADS]
            blk = blk.reshape(P, HEADS, D)            # head-major cols
            pm, jm = np.nonzero(eids >= 0)
            a_full[eids[pm, jm]] = blk[pm, :, jm]
    return out_full, a_full[:, :, None]


def kernel(features, src, dst, W_src, b_src, W_dst, b_dst, attn_w, bias,
           gamma, beta, _want_profile=False):
    features = np.asarray(features, np.float32)
    prep = _prep(features, src, dst, W_src, b_src, W_dst, b_dst, attn_w,
                 bias, gamma, beta)
    key = (prep["T"], prep["d_sched"])
    if key not in _COMPILE_CACHE:
        _COMPILE_CACHE[key] = build_program(prep["T"], prep["d_sched"],
                                            prep["S"])
    nc = _COMPILE_CACHE[key]
    res = run_bass_kernel_spmd(nc, prep["core_inputs"],
                               core_ids=list(range(NCORES)),
                               trace=_want_profile)
    out = _unshard(prep, res.results)
    if _want_profile:
        kernel.last_results = res
    return out


if __name__ == "__main__":
    # tiny self-check with a small random graph on the real reference math
    import jax
    jax.config.update("jax_platforms", "cpu")
    rng = np.random.default_rng(0)
    N, E = 50000, 800000
    feats = rng.standard_normal((N, 256), dtype=np.float32)
    s = rng.integers(0, N, E).astype(np.int64)
    d = rng.integers(0, N, E).astype(np.int64)
    Ws = (rng.standard_normal((256, 256)) * 0.05).astype(np.float32)
    Wd = (rng.standard_normal((256, 256)) * 0.05).astype(np.float32)
    aw = (rng.standard_normal((4, 64)) * 0.05).astype(np.float32)
    z = np.zeros(256, np.float32)
    o = np.ones(256, np.float32)
    out, a = kernel(feats, s, d, Ws, z, Wd, z, aw, z, o, z)
    print("kernel ran:", out.shape, a.shape)
